# Initial kernel scaffold
#
"""Conditional InstanceNorm1D on 8 Trainium2 NeuronCores.

x: [32, 256, 8192] f32. Per-(b, c) instance norm over L (biased var), then a
per-sample style affine: y = x_hat * weight[style_ids[b], c] + bias[style_ids[b], c].

Sharding: pure data parallel over batch. Each core gets 4 samples ->
1024 (b, c) rows of length 8192, processed as 8 tiles of [128 partitions, 8192].
The tiny [S, C] style tables are gathered host-side into per-row scale/shift
columns so the device kernel has no indirect addressing.

Per tile the device does:
  mean/var  : 16x bn_stats (512-elem subgroups) + bn_aggr       (VectorE)
  rstd      : sqrt(var + eps) on ScalarE, reciprocal on VectorE
  fold      : sc = rstd * w_row ; sh = b_row - mean * sc        (VectorE, [128,1])
  apply     : y = Identity(sc * x + sh) in place                (ScalarE)
Loads are issued on the sync sequencer (HWDGE), stores on the scalar
sequencer (HWDGE) so load and store issue never serialize on one queue.
"""

import numpy as np

import concourse.bass as bass
import concourse.tile as tile
from concourse import mybir
from concourse.bass_utils import run_bass_kernel_spmd

B, C, L, S = 32, 256, 8192, 4
N_CORES = 8
B_PER = B // N_CORES            # 4 samples per core
ROWS = B_PER * C                # 1024 (b, c) rows per core
P = 128                         # SBUF partitions
EPS = 1e-5
F32 = mybir.dt.float32
BN_FMAX = 512                   # bn_stats free-dim hardware limit


def build_nc(rows: int = ROWS, length: int = L, xbufs: int = 4) -> bass.Bass:
    ntiles = rows // P
    nsub = length // BN_FMAX

    nc = bass.Bass()
    x_d = nc.dram_tensor("x", [rows, length], F32, kind="ExternalInput")
    w_d = nc.dram_tensor("w", [P, ntiles], F32, kind="ExternalInput")
    b_d = nc.dram_tensor("b", [P, ntiles], F32, kind="ExternalInput")
    y_d = nc.dram_tensor("y", [rows, length], F32, kind="ExternalOutput")

    with tile.TileContext(nc) as tc:
        with (
            tc.tile_pool(name="xp", bufs=xbufs) as xp,
            tc.tile_pool(name="consts", bufs=1) as consts,
            tc.tile_pool(name="stats", bufs=3) as stats,
        ):
            wt = consts.tile([P, ntiles], F32)
            bt = consts.tile([P, ntiles], F32)
            nc.sync.dma_start(out=wt[:], in_=w_d[:])
            nc.sync.dma_start(out=bt[:], in_=b_d[:])

            for i in range(ntiles):
                xt = xp.tile([P, length], F32)
                nc.sync.dma_start(out=xt[:], in_=x_d[i * P:(i + 1) * P, :])

                xr = xt.rearrange("p (n f) -> p n f", f=BN_FMAX)
                st = stats.tile([P, nsub, 6], F32)
                for j in range(nsub):
                    nc.vector.bn_stats(out=st[:, j, :], in_=xr[:, j, :])
                mv = stats.tile([P, 2], F32)
                nc.vector.bn_aggr(out=mv[:], in_=st[:])

                sc = stats.tile([P, 1], F32)
                sh = stats.tile([P, 1], F32)
                # sc = weight_row / sqrt(var + eps); sh = bias_row - mean * sc
                nc.scalar.activation(
                    out=sc[:], in_=mv[:, 1:2],
                    func=mybir.ActivationFunctionType.Sqrt, bias=EPS,
                )
                nc.vector.reciprocal(out=sc[:], in_=sc[:])
                nc.vector.tensor_mul(sc[:], sc[:], wt[:, i:i + 1])
                nc.vector.tensor_mul(sh[:], mv[:, 0:1], sc[:])
                nc.vector.tensor_sub(sh[:], bt[:, i:i + 1], sh[:])

                # y = sc * x + sh, in place (Identity shares an ACT table set
                # with Sqrt, so no table-switch cost)
                nc.scalar.activation(
                    out=xt[:], in_=xt[:],
                    func=mybir.ActivationFunctionType.Identity,
                    bias=sh[:], scale=sc[:],
                )
                nc.scalar.dma_start(out=y_d[i * P:(i + 1) * P, :], in_=xt[:])
    return nc


_NC = None


def _get_nc() -> bass.Bass:
    global _NC
    if _NC is None:
        _NC = build_nc()
    return _NC


def _shard_inputs(x, weight, bias, style_ids):
    """Host-side prep: gather style tables per sample, split batch across cores."""
    x = np.asarray(x)
    if x.dtype != np.float32:
        x = x.astype(np.float32)
    weight = np.asarray(weight, dtype=np.float32)
    bias = np.asarray(bias, dtype=np.float32)
    sid = np.asarray(style_ids).astype(np.int64)

    w_g = weight[sid]           # [B, C]
    b_g = bias[sid]             # [B, C]
    ntiles = ROWS // P

    in_maps = []
    for m in range(N_CORES):
        xs = np.ascontiguousarray(x[m * B_PER:(m + 1) * B_PER].reshape(ROWS, L))
        # column i of the [P, ntiles] table = rows i*128..(i+1)*128 of the shard
        wg = np.ascontiguousarray(
            w_g[m * B_PER:(m + 1) * B_PER].reshape(ntiles, P).T)
        bg = np.ascontiguousarray(
            b_g[m * B_PER:(m + 1) * B_PER].reshape(ntiles, P).T)
        in_maps.append({"x": xs, "w": wg, "b": bg})
    return in_maps


def run_sharded(x, weight, bias, style_ids, **spmd_kwargs):
    """Shard, run on cores 0-7, gather. Returns (output, BassKernelResults)."""
    in_maps = _shard_inputs(x, weight, bias, style_ids)
    res = run_bass_kernel_spmd(_get_nc(), in_maps, list(range(N_CORES)), **spmd_kwargs)
    out = np.empty((B, C, L), dtype=np.float32)
    for m in range(N_CORES):
        out[m * B_PER:(m + 1) * B_PER] = res.results[m]["y"].reshape(B_PER, C, L)
    return out, res


def kernel(x, weight, bias, style_ids):
    out, _ = run_sharded(x, weight, bias, style_ids)
    return out


# revision 12
# speedup vs baseline: 1.0975x; 1.0975x over previous
"""Conditional InstanceNorm1D on 8 Trainium2 NeuronCores.

x: [32, 256, 8192] f32. Per-(b, c) instance norm over L (biased var), then a
per-sample style affine: y = x_hat * weight[style_ids[b], c] + bias[style_ids[b], c].

Sharding: pure data parallel over batch. Each core gets 4 samples ->
1024 (b, c) rows of length 8192, processed as 8 tiles of [128 partitions, 8192].
The tiny [S, C] style tables are gathered host-side into per-row scale/shift
columns so the device kernel has no indirect addressing.

Per tile the device does:
  mean/var  : 16x bn_stats (512-elem subgroups) + bn_aggr       (VectorE)
  rstd      : sqrt(var + eps) on ScalarE, reciprocal on VectorE
  fold      : sc = rstd * w_row ; sh = b_row - mean * sc        (VectorE, [128,1])
  apply     : y = Identity(sc * x + sh) in place                (ScalarE)
Loads are issued on the sync sequencer (HWDGE), stores on the scalar
sequencer (HWDGE) so load and store issue never serialize on one queue.
"""

import numpy as np

import concourse.bacc as bacc
import concourse.bass as bass
import concourse.tile as tile
from concourse import mybir
from concourse.bass_utils import run_bass_kernel_spmd

B, C, L, S = 32, 256, 8192, 4
N_CORES = 8
B_PER = B // N_CORES            # 4 samples per core
ROWS = B_PER * C                # 1024 (b, c) rows per core
P = 128                         # SBUF partitions
EPS = 1e-5
F32 = mybir.dt.float32
BN_FMAX = 512                   # bn_stats free-dim hardware limit


def build_nc(rows: int = ROWS, length: int = L, xbufs: int = 4,
             reps: int = 1, loop_reps: int = 0) -> bass.Bass:
    """reps > 1 unrolls the whole pass inside one NEFF; loop_reps > 0 wraps
    the pass in a hardware For_i loop (benchmarking only: the
    (T(R2)-T(R1))/(R2-R1) delta cancels the ~90 ms axon dispatch cost)."""
    ntiles = rows // P
    nsub = length // BN_FMAX

    # Bacc (not plain Bass): its finalize() runs generate_event_semaphores,
    # which splits multi-sem waits — TRN2 compute instructions carry at most
    # one sync wait, and walrus rejects the program otherwise.
    nc = bacc.Bacc()
    x_d = nc.dram_tensor("x", [rows, length], F32, kind="ExternalInput")
    w_d = nc.dram_tensor("w", [P, ntiles], F32, kind="ExternalInput")
    b_d = nc.dram_tensor("b", [P, ntiles], F32, kind="ExternalInput")
    y_d = nc.dram_tensor("y", [rows, length], F32, kind="ExternalOutput")

    with tile.TileContext(nc) as tc:
        with (
            tc.tile_pool(name="xp", bufs=xbufs) as xp,
            tc.tile_pool(name="consts", bufs=1) as consts,
            tc.tile_pool(name="stats", bufs=ntiles) as stats,
        ):
            wt_in = consts.tile([P, ntiles], F32)
            bt_in = consts.tile([P, ntiles], F32)
            nc.sync.dma_start(out=wt_in[:], in_=w_d[:])
            nc.sync.dma_start(out=bt_in[:], in_=b_d[:])
            # bounce through a DVE copy: walrus rejects TensorTensor
            # instructions that need a DMA-sem wait (1 wait slot), so make
            # the copy absorb the DMA wait and feed DVE-produced tiles to
            # the per-tile TT ops.
            wt = consts.tile([P, ntiles], F32)
            bt = consts.tile([P, ntiles], F32)
            nc.vector.tensor_copy(wt[:], wt_in[:])
            nc.vector.tensor_copy(bt[:], bt_in[:])
            eps_t = consts.tile([P, 1], F32)
            nc.vector.memset(eps_t[:], EPS)

            def emit_body():
                for i in range(ntiles * reps):
                    i = i % ntiles
                    xt = xp.tile([P, length], F32)
                    nc.sync.dma_start(out=xt[:], in_=x_d[i * P:(i + 1) * P, :])

                    xr = xt.rearrange("p (n f) -> p n f", f=BN_FMAX)
                    st = stats.tile([P, nsub, 6], F32)
                    for j in range(nsub):
                        nc.vector.bn_stats(out=st[:, j, :], in_=xr[:, j, :])
                    mv = stats.tile([P, 2], F32)
                    nc.vector.bn_aggr(out=mv[:], in_=st[:])

                    sc = stats.tile([P, 1], F32)
                    sh = stats.tile([P, 1], F32)
                    # sc = weight_row / sqrt(var + eps); sh = bias_row - mean * sc
                    nc.scalar.activation(
                        out=sc[:], in_=mv[:, 1:2],
                        func=mybir.ActivationFunctionType.Sqrt, bias=eps_t[:],
                    )
                    nc.vector.reciprocal(out=sc[:], in_=sc[:])
                    nc.vector.tensor_mul(sc[:], sc[:], wt[:, i:i + 1])
                    nc.vector.tensor_mul(sh[:], mv[:, 0:1], sc[:])
                    nc.vector.tensor_sub(sh[:], bt[:, i:i + 1], sh[:])

                    # y = sc * x + sh, in place (Identity shares an ACT table
                    # set with Sqrt, so no table-switch cost)
                    nc.scalar.activation(
                        out=xt[:], in_=xt[:],
                        func=mybir.ActivationFunctionType.Identity,
                        bias=sh[:], scale=sc[:],
                    )
                    nc.scalar.dma_start(out=y_d[i * P:(i + 1) * P, :], in_=xt[:])

            if loop_reps:
                with tc.For_i(0, loop_reps, 1) as _it:
                    emit_body()
            else:
                emit_body()
    nc.finalize()
    return nc


_NC = None


def _get_nc() -> bass.Bass:
    global _NC
    if _NC is None:
        _NC = build_nc()
    return _NC


def _shard_inputs(x, weight, bias, style_ids):
    """Host-side prep: gather style tables per sample, split batch across cores."""
    x = np.asarray(x)
    if x.dtype != np.float32:
        x = x.astype(np.float32)
    weight = np.asarray(weight, dtype=np.float32)
    bias = np.asarray(bias, dtype=np.float32)
    sid = np.asarray(style_ids).astype(np.int64)

    w_g = weight[sid]           # [B, C]
    b_g = bias[sid]             # [B, C]
    ntiles = ROWS // P

    in_maps = []
    for m in range(N_CORES):
        xs = np.ascontiguousarray(x[m * B_PER:(m + 1) * B_PER].reshape(ROWS, L))
        # column i of the [P, ntiles] table = rows i*128..(i+1)*128 of the shard
        wg = np.ascontiguousarray(
            w_g[m * B_PER:(m + 1) * B_PER].reshape(ntiles, P).T)
        bg = np.ascontiguousarray(
            b_g[m * B_PER:(m + 1) * B_PER].reshape(ntiles, P).T)
        in_maps.append({"x": xs, "w": wg, "b": bg})
    return in_maps


def run_sharded(x, weight, bias, style_ids, **spmd_kwargs):
    """Shard, run on cores 0-7, gather. Returns (output, BassKernelResults)."""
    in_maps = _shard_inputs(x, weight, bias, style_ids)
    res = run_bass_kernel_spmd(_get_nc(), in_maps, list(range(N_CORES)), **spmd_kwargs)
    out = np.empty((B, C, L), dtype=np.float32)
    for m in range(N_CORES):
        out[m * B_PER:(m + 1) * B_PER] = res.results[m]["y"].reshape(B_PER, C, L)
    return out, res


def kernel(x, weight, bias, style_ids):
    out, _ = run_sharded(x, weight, bias, style_ids)
    return out
